# revision 18
# baseline (speedup 1.0000x reference)
"""KeOps-style multi-head attention (unnormalized-exp softmax) on 8 trn2 cores.

Sharding: core c handles batch bi = c//2 and query rows u*1024..(u+1)*1024
(u = c%2), ALL 8 heads. Output is a pure concat over cores (no reduction).

All matmul operands are bf16 (PE runs 1 cycle/row vs 4 for fp32); every
accumulation (PSUM), the softmax denominator path and the final output stay
fp32, keeping the end-to-end rel error ~5e-3 (gate is 2e-2).

Per-core pipeline (one uniform SPMD program):
  A) Batched bf16 DMAs (x via the Activation HWDGE queue, weights/xq via SP);
     transpose x on PE (128x128 identity-matmul transposes, bf16 1 cyc/row),
     8 transposes batched per DVE evacuation.
  B) QKV projections producing q^T/k^T in "stacked head" layout
     [32*h_local + d, n] (heads packed 4-per-tensor so the K=32 scores
     matmuls can be row-tiled on the PE array), and v in normal layout with a
     ones-column appended (the softmax denominator falls out of the same
     matmul that computes the numerator). Only the i=0 half and v chunks
     0..7 are projected in the prologue; the rest interleave into attention
     blocks 1-2 just-in-time on borrowed score-PSUM slots.
  C) Attention over 4 blocks (2 query groups x 2 head quads), software-
     pipelined by one key chunk: scores(j) [4x row-tiled K=32 bf16 matmuls,
     each into its own PSUM bank] -> exp(j) on ACT (psum -> sbuf bf16,
     [128,1024] per instruction) -> numer(j-1) accumulation (e^T moving).
     Numerator accumulators pack two heads per PSUM bank ([97, 512], heads
     at partitions 0/64 - the hardware overlaps those column-group-disjoint
     matmuls) and are double-buffered, so evacuation (DVE + DMA assembly of
     PT) spreads one-copy-per-j through the NEXT block with no boundary
     stall. exp can optionally run partially on DVE via a one-instruction
     Schraudolph bf16 exp (DVE_PICK residues); with the PE clock-limited it
     buys nothing, so it is off for the better accuracy.
  D) Per-query-group epilogue fully overlapped with the next group's
     compute: reciprocal of the denominators in 4 small DVE chunks,
     partition-broadcast via a DRAM bounce, normalize multiplies on the
     otherwise-idle GpSimd (emits bf16 PT), and the output projection on
     borrowed score-PSUM slots; only the second group's short chain remains
     as the tail.

Known limit: the attention loop's matmuls (32-row scores, 33-col numerators)
never look "busy" to the PE_HAM activity monitor, so the PE clock-gate sits
at K=4/8 (1.2 GHz) for most of the loop; full-array dummy work flips it 8/8
only in bursts and costs more than it buys. The loop is therefore PE-bound
at ~537ns per concurrent matmul pair.
"""

import numpy as np
import ml_dtypes
from contextlib import ExitStack

import concourse.bass as bass
import concourse.mybir as mybir
import concourse.tile as tile
from concourse import bacc
from concourse.bass_utils import run_bass_kernel_spmd

DIM = 256
NUM_HEADS = 8
HEAD_DIM = 32
B = 4
N = 2048
NQ = 1024          # query rows per core
NCORES = 8
FP = mybir.dt.float32
BF = mybir.dt.bfloat16
EXP = mybir.ActivationFunctionType.Exp
CPY = mybir.ActivationFunctionType.Copy
LN = mybir.ActivationFunctionType.Ln
I16 = mybir.dt.int16
MULT = mybir.AluOpType.mult
ADD = mybir.AluOpType.add

NT_KV = N // 128   # 16 n-tiles of kv rows
NT_Q = NQ // 128   # 8 n-tiles of q rows
NGQ = NQ // 512    # 2 groups of 512 query cols in q^T
NJ = N // 128      # 16 key chunks of 128
DUMMY_MM_ROWS = 0  # rows of full-array dummy matmul per j (0=off; HAM
DUMMY_EVERY = 2    # experiment knob - net loss, the clock boost never paid)
SCORE_MODE = "pad64"  # "pair32": original 32-row row-tiled pairs (HAM
                      #   throttles the PE clock to 4/8 for the whole loop);
                      # "full128": kT whole tile as lhsT vs zero-padded
                      #   per-head q rhs - math identical, HAM sees full-
                      #   array matmuls and holds K=8/8 (-14us) but the 4
                      #   score matmuls serialize;
                      # "pad64": 64-row bands (head + zero-padded sibling),
                      #   two matmuls pair on disjoint row halves -> both
                      #   overlap AND (hopefully) enough activity for K=8/8.
DVE_PICK = set()   # (2j+p)%8 residues whose exp runs on DVE (Schraudolph)
SCHRAU_A = float(2 ** 7 / np.log(2.0))
SCHRAU_B = float(127 * 2 ** 7 - 8.0)


def build_program():
    nc = bacc.Bacc()

    xq = nc.declare_dram_parameter("xq", [NQ, DIM], BF, isOutput=False)
    xkv = nc.declare_dram_parameter("xkv", [N, DIM], BF, isOutput=False)
    wq = nc.declare_dram_parameter("wq", [DIM, DIM], BF, isOutput=False)
    wk = nc.declare_dram_parameter("wk", [DIM, DIM], BF, isOutput=False)
    wv = nc.declare_dram_parameter("wv", [DIM, DIM], BF, isOutput=False)
    wout = nc.declare_dram_parameter("wout", [DIM, DIM], BF, isOutput=False)
    bout = nc.declare_dram_parameter("bout", [DIM], FP, isOutput=False)
    ident_in = nc.declare_dram_parameter("ident", [128, 128], BF, isOutput=False)
    out = nc.declare_dram_parameter("out", [NQ, DIM], FP, isOutput=True)

    with tile.TileContext(nc) as tc, ExitStack() as ctx:
        consts = ctx.enter_context(tc.tile_pool(name="consts", bufs=1))
        persist = ctx.enter_context(tc.tile_pool(name="persist", bufs=1))

        ident = consts.tile([128, 128], BF)
        nc.sync.dma_start(out=ident, in_=ident_in[:, :])
        bias_b = consts.tile([128, DIM], FP)

        # ---- x loads: 8-block batched DMAs (the SP sequencer issues DMA
        # instructions at ~600ns each, so fewer+bigger wins; xkv rides the
        # Activation queue, which is also HWDGE-capable and idle here; xq
        # goes ahead of the weights because the q projection gates the loop
        # start) ----
        xkv_sb = [persist.tile([128, 4, DIM], BF, tag=f"xkvb{b}", name=f"xkvb{b}")
                  for b in range(4)]
        for b in range(4):
            nc.scalar.dma_start(
                out=xkv_sb[b],
                in_=xkv[512 * b:512 * (b + 1), :]
                .rearrange("(t p) d -> p t d", p=128))
        # ---- weights (wq/wk first: the q/k projections gate the loop
        # start; bias is only needed by the output projection) ----
        wq_sb = consts.tile([128, 2, DIM], BF)
        wk_sb = consts.tile([128, 2, DIM], BF)
        wv_sb = consts.tile([128, 2, DIM], BF)
        wout_sb = consts.tile([128, 2, DIM], BF)
        for w_sb, w_d in ((wq_sb, wq), (wk_sb, wk)):
            nc.sync.dma_start(
                out=w_sb, in_=w_d[:, :].rearrange("(ck p) d -> p ck d", p=128))
        xq_sb = [persist.tile([128, 4, DIM], BF, tag=f"xqb{b}", name=f"xqb{b}")
                 for b in range(2)]
        for b in range(2):
            nc.sync.dma_start(
                out=xq_sb[b],
                in_=xq[512 * b:512 * (b + 1), :]
                .rearrange("(t p) d -> p t d", p=128))
        for w_sb, w_d in ((wv_sb, wv), (wout_sb, wout)):
            nc.sync.dma_start(
                out=w_sb, in_=w_d[:, :].rearrange("(ck p) d -> p ck d", p=128))
        nc.sync.dma_start(out=bias_b, in_=bout[:].unsqueeze(0).to_broadcast([128, DIM]))
        # Warm the ACT table RAM with the exp set during the prologue so
        # the first real exp doesn't stall ~2.7us on ACT_TABLE_LOAD.
        tbl_warm = consts.tile([1, DIM], FP)
        nc.scalar.activation(tbl_warm, bias_b[0:1, :], EXP)

        # ---- transposes: xkvT [128(c_local), ck, n], xqT [128, ck, nq] ----
        # 8 transposes share one [128, 1024] bf16 psum tile (sequential
        # writes to one bank), evacuated by a single DVE copy.
        xkvT = persist.tile([128, 2, N], BF)
        xqT = persist.tile([128, 2, NQ], BF)
        with tc.tile_pool(name="tps", bufs=3, space="PSUM") as tps:
            for ck in range(2):
                for b8 in range(NT_KV // 8):
                    ps = tps.tile([128, 1024], BF, tag="tp")
                    for k in range(8):
                        t = 8 * b8 + k
                        nc.tensor.transpose(
                            ps[:, 128 * k:128 * (k + 1)],
                            xkv_sb[t // 4][:, t % 4, 128 * ck:128 * (ck + 1)], ident)
                    nc.vector.tensor_copy(
                        xkvT[:, ck, 1024 * b8:1024 * (b8 + 1)], ps)
                ps = tps.tile([128, 1024], BF, tag="tp")
                for k in range(NT_Q):
                    nc.tensor.transpose(
                        ps[:, 128 * k:128 * (k + 1)],
                        xq_sb[k // 4][:, k % 4, 128 * ck:128 * (ck + 1)], ident)
                nc.vector.tensor_copy(xqT[:, ck, :], ps)

        # ---- QKV projections ----
        # qT/kT stacked-head layout: tensor i in {0,1} holds heads 4i..4i+3:
        # row 32*hloc + d  <->  head 4i+hloc, dim d.
        qT = [persist.tile([128, NQ], BF, tag=f"qT{i}", name=f"qT{i}") for i in range(2)]
        kT = [persist.tile([128, N], BF, tag=f"kT{i}", name=f"kT{i}") for i in range(2)]
        # v normal layout + ones column: [128(n), t, h, 33]
        v_sb = persist.tile([128, NT_KV, NUM_HEADS, HEAD_DIM + 1], BF)
        nc.vector.memset(v_sb[:, :, :, HEAD_DIM:], 1.0)

        with (
            tc.tile_pool(name="qkvp", bufs=3, space="PSUM") as qkvp,
            tc.tile_pool(name="vp", bufs=2, space="PSUM") as vp,
        ):
            for i in range(1):
                for g in range(NGQ):
                    ps = qkvp.tile([128, 512], FP, tag="proj")
                    for ck in range(2):
                        nc.tensor.matmul(
                            ps, lhsT=wq_sb[:, ck, 128 * i:128 * (i + 1)],
                            rhs=xqT[:, ck, 512 * g:512 * (g + 1)],
                            start=(ck == 0), stop=(ck == 1))
                    nc.vector.tensor_copy(qT[i][:, 512 * g:512 * (g + 1)], ps)
                for g in range(N // 512):
                    ps = qkvp.tile([128, 512], FP, tag="proj")
                    for ck in range(2):
                        nc.tensor.matmul(
                            ps, lhsT=wk_sb[:, ck, 128 * i:128 * (i + 1)],
                            rhs=xkvT[:, ck, 512 * g:512 * (g + 1)],
                            start=(ck == 0), stop=(ck == 1))
                    nc.vector.tensor_copy(kT[i][:, 512 * g:512 * (g + 1)], ps)
            for t in range(NT_KV // 2):
                ps = vp.tile([128, DIM], FP, tag="vproj")
                for ck in range(2):
                    nc.tensor.matmul(
                        ps, lhsT=xkvT[:, ck, 128 * t:128 * (t + 1)],
                        rhs=wv_sb[:, ck, :],
                        start=(ck == 0), stop=(ck == 1))
                # strided copy into the 33-wide per-head slots, on the Scalar
                # engine (idle during the prologue; DVE handles q/k evacs)
                nc.scalar.activation(v_sb[:, t, :, 0:HEAD_DIM], ps, CPY)

        # ---- attention ----
        # PT: normalized pre-projection, transposed: tensor i rows = wout rows
        # 128i..128i+128 (head 4i+hloc dim d at partition 32*hloc+d).
        PTf = [persist.tile([128, NQ], FP, tag=f"PTf{i}", name=f"PTf{i}") for i in range(2)]
        PTb = [persist.tile([128, NQ], BF, tag=f"PTb{i}", name=f"PTb{i}") for i in range(2)]
        # per-(g,hh) denominators ([4, 512], row hloc): separate tiles so
        # each hh half can be reciprocal'd/broadcast as soon as its own
        # evac lands (engine reads must start at an aligned partition, so
        # row-slices of one bigger tile won't do).
        denoms = [[persist.tile([4, 512], FP, tag=f"dn{g}{hh}", name=f"dn{g}{hh}")
                   for hh in range(2)] for g in range(NGQ)]
        recips = [[persist.tile([4, 512], FP, tag=f"rc{g}{hh}", name=f"rc{g}{hh}")
                   for hh in range(2)] for g in range(NGQ)]
        rb = [persist.tile([128, NQ], FP, tag=f"rb{i}", name=f"rb{i}") for i in range(2)]
        if SCORE_MODE != "pair32":
            # per-(hh,hloc) zero-padded q: only rows holding head 4hh+hloc
            # are live (refreshed per g); the zero rows make a wider-K
            # matmul against a taller kT slice compute exactly that head.
            # full128: one [128,512] tile per hloc (band at 32*hloc).
            # pad64: one [128,512] tile per uu; rows 0:64 serve hloc=uu
            # (band at 32*uu) and rows 64:128 serve hloc=2+uu (band at
            # 64+32*uu) - rhs slices start at the same partition as the
            # kT lhsT slice (a hard PE requirement).
            nqp = 4 if SCORE_MODE == "full128" else 2
            qpad = [[persist.tile([128, 512], BF, tag=f"qp{hh}{hl}",
                                  name=f"qp{hh}{hl}") for hl in range(nqp)]
                    for hh in range(2)]
            for hh in range(2):
                for hl in range(nqp):
                    nc.vector.memset(qpad[hh][hl], 0.0)

        with (
            tc.tile_pool(name="spsum", bufs=2, space="PSUM") as spsum,
            tc.tile_pool(name="npsum", bufs=2, space="PSUM") as npsum,
            tc.tile_pool(name="esb", bufs=4) as esb,
            tc.tile_pool(name="evac", bufs=4) as evac,
            tc.tile_pool(name="osb", bufs=4) as osb,
            tc.tile_pool(name="dscratch", bufs=2, space="DRAM") as dsc,
        ):
            # Numerator accumulators pack two heads per PSUM bank ([97, 512]:
            # head pair at partition 0 and 64, both 32-aligned), so a double-
            # buffered pair of tiles fits beside the score tiles.  With
            # bufs=2 the evacuation of block b overlaps block b+1 freely.
            def outproj_t(t, pool):
                if pool is spsum:
                    ps = pool.tile([128, 1024], FP, tag="sp", name=f"op{t}")
                else:
                    ps = pool.tile([128, DIM], FP, tag="o", name=f"op{t}")
                pso = ps[:, 0:DIM]
                for i in range(2):
                    nc.tensor.matmul(
                        pso, lhsT=PTb[i][:, 128 * t:128 * (t + 1)],
                        rhs=wout_sb[:, i, :],
                        start=(i == 0), stop=(i == 1))
                ob = osb.tile([128, DIM], FP, tag="ob")
                nc.vector.tensor_add(ob, pso, bias_b)
                nc.sync.dma_start(out=out[128 * t:128 * (t + 1), :], in_=ob)

            # Side work is spread one closure per j so no engine queue
            # ever receives a multi-us blob ahead of exp work it gates.
            def evac_one(g, hh, nptiles, hloc, on_act=False, tail=False):
                def f():
                    npt = nptiles[hloc // 2]
                    r0 = 64 * (hloc % 2)
                    tmp = evac.tile([HEAD_DIM + 1, 512], FP, tag="ev")
                    if on_act:
                        nc.scalar.activation(tmp, npt[r0:r0 + HEAD_DIM + 1, :],
                                             CPY)
                    else:
                        nc.vector.tensor_copy(tmp, npt[r0:r0 + HEAD_DIM + 1, :])
                    nc.sync.dma_start(
                        out=PTf[hh][32 * hloc:32 * hloc + 32,
                                    512 * g:512 * (g + 1)],
                        in_=tmp[0:HEAD_DIM, :])
                    # in the tail the GpSimd queue is idle - issue the denom
                    # row DMAs there so the SP queue only carries the PTf rows
                    eng = nc.gpsimd if tail else nc.sync
                    eng.dma_start(
                        out=denoms[g][hh][hloc:hloc + 1, :],
                        in_=tmp[HEAD_DIM:HEAD_DIM + 1, :])
                return f

            def recip_hh(g, hh):
                # ~18 correct bits, 5x faster than the Newton reciprocal()
                # (denominators are sums of exps, comfortably inside range)
                def f():
                    nc.vector.reciprocal_approx_fast(recips[g][hh],
                                                     denoms[g][hh])
                return f

            def bcast_hh(g, hh, tail=False):
                # In the tail, half the broadcasts issue from the ACT HWDGE
                # queue in parallel with the SP queue (~590ns sequencer issue
                # per dma_start dominates the tail chain). Mid-loop
                # they stay on SP: the ACT queue is busy with exps there.
                def f():
                    rd = dsc.tile([4, 512], FP, tag="rd")
                    nc.sync.dma_start(out=rd[:, :], in_=recips[g][hh])
                    for hloc in range(4):
                        eng = nc.scalar if (tail and hloc % 2 == 0) else nc.sync
                        eng.dma_start(
                            out=rb[hh][32 * hloc:32 * hloc + 32,
                                       512 * g:512 * (g + 1)],
                            in_=rd[hloc:hloc + 1, :]
                            .to_broadcast([32, 512]))
                return f

            def norm_mul(g, hh):
                def f():
                    nc.gpsimd.tensor_mul(
                        PTb[hh][:, 512 * g:512 * (g + 1)],
                        PTf[hh][:, 512 * g:512 * (g + 1)],
                        rb[hh][:, 512 * g:512 * (g + 1)])
                return f

            def oproj(t):
                def f():
                    outproj_t(t, spsum)
                return f

            def proj1(kind, g):
                # i=1 q/k projection group on a borrowed sp PSUM slot:
                # real full-array PE work interleaved into blocks 1-2
                # just-in-time (shorter prologue, and the HAM activity
                # monitor sees a busy full array -> clock stays up longer)
                def f():
                    ps = spsum.tile([128, 1024], FP, tag="sp", name=f"pj{kind}{g}")
                    pso = ps[:, 0:512]
                    w_sb, dstT, srcT = ((wq_sb, qT, xqT) if kind == "q"
                                        else (wk_sb, kT, xkvT))
                    for ck in range(2):
                        nc.tensor.matmul(
                            pso, lhsT=w_sb[:, ck, 128:256],
                            rhs=srcT[:, ck, 512 * g:512 * (g + 1)],
                            start=(ck == 0), stop=(ck == 1))
                    nc.vector.tensor_copy(dstT[1][:, 512 * g:512 * (g + 1)], pso)
                return f

            def vpair(t):
                # v projection for chunks t, t+1 on a borrowed sp PSUM slot
                def f():
                    ps = spsum.tile([128, 1024], FP, tag="sp", name=f"vp{t}")
                    for w in range(2):
                        pso = ps[:, 256 * w:256 * (w + 1)]
                        for ck in range(2):
                            nc.tensor.matmul(
                                pso,
                                lhsT=xkvT[:, ck, 128 * (t + w):128 * (t + w + 1)],
                                rhs=wv_sb[:, ck, :],
                                start=(ck == 0), stop=(ck == 1))
                    nc.scalar.activation(v_sb[:, t, :, 0:HEAD_DIM],
                                         ps[:, 0:256], CPY)
                    nc.vector.tensor_copy(v_sb[:, t + 1, :, 0:HEAD_DIM],
                                          ps[:, 256:512])
                return f

            sidework = {
                (0, 0): [None] * 16,
                (0, 1): [None] * 16,
                # during (g1, hh0): evac (g0,hh1); normalize g0; outproj g0
                (1, 0): [None] * 16,
                (1, 1): [None] * 16,
            }
            w00 = sidework[(0, 0)]
            w00[1] = proj1("q", 0)
            w00[3] = proj1("q", 1)
            w00[5] = proj1("k", 0)
            for c in range(4):
                w00[2 * c + 2] = vpair(8 + 2 * c)
            w01 = sidework[(0, 1)]
            w01[1] = proj1("k", 1)
            w01[4] = proj1("k", 2)
            w01[8] = proj1("k", 3)
            for g in range(NGQ):
                for hh in range(2):
                    if SCORE_MODE == "full128":
                        for hl in range(4):
                            nc.vector.tensor_copy(
                                qpad[hh][hl][32 * hl:32 * hl + 32, :],
                                qT[hh][32 * hl:32 * hl + 32,
                                       512 * g:512 * (g + 1)])
                    elif SCORE_MODE == "pad64":
                        for hl in range(4):
                            uu = hl % 2
                            r0 = 64 * (hl // 2) + 32 * uu
                            nc.vector.tensor_copy(
                                qpad[hh][uu][r0:r0 + 32, :],
                                qT[hh][32 * hl:32 * hl + 32,
                                       512 * g:512 * (g + 1)])
                    nptiles = [npsum.tile([97, 512], FP, tag=f"np{x}",
                                          name=f"np{x}") for x in range(2)]

                    def numers(j, es):
                        for p in range(2):
                            for uu in range(2):
                                hloc = 2 * p + uu
                                h = 4 * hh + hloc
                                npt = nptiles[hloc // 2]
                                r0 = 64 * (hloc % 2)
                                nc.tensor.matmul(
                                    npt[r0:r0 + HEAD_DIM + 1, :],
                                    lhsT=v_sb[:, j, h, :],
                                    rhs=es[p][:, 512 * uu:512 * (uu + 1)],
                                    start=(j == 0), stop=(j == NJ - 1))

                    # Software-pipelined by one j: scores(j) [4x row-tiled,
                    # all concurrent] -> exp(j) -> numer(j-1), so the PE
                    # fills the exp latency with independent score work.
                    # exp runs on ACT except for DVE_PICK residues, which use
                    # a one-instruction Schraudolph exp on DVE:
                    #   bf16(exp(x)) ~= bitcast_bf16(int16(x*128/ln2 + Bc))
                    prev = None
                    work = sidework.get((g, hh))
                    for j in range(NJ):
                        sps = []
                        for p in range(2):
                            sp = spsum.tile([128, 1024], FP, tag="sp")
                            if p == 0 and DUMMY_MM_ROWS and j % DUMMY_EVERY == 0:
                                nc.tensor.matmul(
                                    sp[:, 0:DUMMY_MM_ROWS],
                                    lhsT=wout_sb[:, 0, 0:128],
                                    rhs=xkvT[:, 0, 0:DUMMY_MM_ROWS],
                                    start=True, stop=True)
                            sps.append(sp)
                        # pad64: emit in hloc order 0,2,1,3 so adjacent
                        # matmuls sit on disjoint PE row halves and pair.
                        hlocs = (0, 2, 1, 3) if SCORE_MODE == "pad64" else (0, 1, 2, 3)
                        for hloc in hlocs:
                            p, uu = hloc // 2, hloc % 2
                            sp = sps[p]
                            r = 32 * hloc
                            if SCORE_MODE == "full128":
                                nc.tensor.matmul(
                                    sp[:, 512 * uu:512 * (uu + 1)],
                                    lhsT=kT[hh][:, 128 * j:128 * (j + 1)],
                                    rhs=qpad[hh][hloc],
                                    start=True, stop=True)
                            elif SCORE_MODE == "pad64":
                                r64 = 64 * (hloc // 2)
                                nc.tensor.matmul(
                                    sp[:, 512 * uu:512 * (uu + 1)],
                                    lhsT=kT[hh][r64:r64 + 64,
                                                128 * j:128 * (j + 1)],
                                    rhs=qpad[hh][uu][r64:r64 + 64, :],
                                    start=True, stop=True,
                                    tile_position=(r64, 0))
                            else:
                                nc.tensor.matmul(
                                    sp[:, 512 * uu:512 * (uu + 1)],
                                    lhsT=kT[hh][r:r + 32, 128 * j:128 * (j + 1)],
                                    rhs=qT[hh][r:r + 32, 512 * g:512 * (g + 1)],
                                    start=True, stop=True,
                                    tile_position=(r, 0))
                        es = []
                        for p in range(2):
                            e = esb.tile([128, 1024], BF, tag="e")
                            if (2 * j + p) % 8 in DVE_PICK:
                                nc.vector.tensor_scalar(
                                    out=e.bitcast(I16), in0=sps[p],
                                    scalar1=SCHRAU_A, scalar2=SCHRAU_B,
                                    op0=MULT, op1=ADD)
                            else:
                                nc.scalar.activation(e, sps[p], EXP)
                            es.append(e)
                        if prev is not None:
                            numers(*prev)
                        if work is not None and work[j] is not None:
                            work[j]()
                        prev = (j, es)
                    numers(*prev)
                    if (g, hh) == (0, 0):
                        # block (0,1) hosts evac(0,0) AND the whole hh=0
                        # normalize of g=0 (its denoms are complete here)
                        w = sidework[(0, 1)]
                        for slot, k in zip((0, 2, 3, 5), range(4)):
                            w[slot] = evac_one(0, 0, nptiles, k)
                        w[7] = recip_hh(0, 0)
                        w[9] = bcast_hh(0, 0)
                        w[11] = norm_mul(0, 0)
                    elif (g, hh) == (0, 1):
                        # block (1,0): finish g=0 (hh=1 normalize) and run
                        # ALL four g=0 output projections in-loop
                        w = sidework[(1, 0)]
                        for k in range(4):
                            w[k] = evac_one(0, 1, nptiles, k)
                        w[5] = recip_hh(0, 1)
                        w[6] = bcast_hh(0, 1)
                        w[8] = norm_mul(0, 1)
                        w[10] = oproj(0)
                        w[12] = oproj(1)
                        w[13] = oproj(2)
                        w[15] = oproj(3)
                    elif (g, hh) == (1, 0):
                        # block (1,1): evac(1,0) plus the hh=0 half of the
                        # g=1 normalize, so the tail only owns hh=1
                        w = sidework[(1, 1)]
                        for k in range(4):
                            w[k] = evac_one(1, 0, nptiles, k)
                        w[5] = recip_hh(1, 0)
                        w[7] = bcast_hh(1, 0)
                        w[9] = norm_mul(1, 0)
                    else:
                        last_np = nptiles
            # tail: last quad's evac + hh=1 normalize + remaining outproj.
            # (An ACT ln->exp reciprocal was tried here and reverted:
            # having Ln in the program forces extra ACT table loads that
            # eat the gain.)
            for k in range(4):
                evac_one(1, 1, last_np, k, on_act=(k % 2 == 0), tail=True)()
            recip_hh(1, 1)()
            bcast_hh(1, 1, tail=True)()
            nc.vector.tensor_mul(PTb[1][:, 512:1024],
                                 PTf[1][:, 512:1024], rb[1][:, 512:1024])
            for t in range(4, NT_Q):
                outproj_t(t, spsum)
    if not nc.is_finalized():
        nc.finalize()
    return nc


_NC_CACHE = None


def _get_program():
    global _NC_CACHE
    if _NC_CACHE is None:
        _NC_CACHE = build_program()
    return _NC_CACHE


def kernel(x, Wqkv, Wout, bout, _trace=False, _trace_kwargs=None):
    bf = ml_dtypes.bfloat16
    x = np.asarray(x, dtype=np.float32)
    Wqkv = np.asarray(Wqkv, dtype=np.float32)
    Wout = np.asarray(Wout, dtype=np.float32)
    bout = np.asarray(bout, dtype=np.float32)

    scale = HEAD_DIM ** -0.5
    wq = np.ascontiguousarray(Wqkv[:, 0:DIM] * scale).astype(bf)
    wk = np.ascontiguousarray(Wqkv[:, DIM:2 * DIM]).astype(bf)
    wv = np.ascontiguousarray(Wqkv[:, 2 * DIM:3 * DIM]).astype(bf)
    wo = np.ascontiguousarray(Wout).astype(bf)
    xb = x.astype(bf)

    in_maps = []
    for c in range(NCORES):
        bi, u = c // 2, c % 2
        in_maps.append({
            "xq": np.ascontiguousarray(xb[bi, u * NQ:(u + 1) * NQ, :]),
            "xkv": np.ascontiguousarray(xb[bi]),
            "wq": wq, "wk": wk, "wv": wv,
            "wout": wo,
            "bout": bout,
            "ident": np.eye(128, dtype=np.float32).astype(bf),
        })

    nc = _get_program()
    kwargs = {}
    if _trace:
        kwargs["trace"] = True
        if _trace_kwargs:
            kwargs.update(_trace_kwargs)
    res = run_bass_kernel_spmd(nc, in_maps, core_ids=list(range(NCORES)), **kwargs)

    outf = np.empty((B, N, DIM), dtype=np.float32)
    for c in range(NCORES):
        bi, u = c // 2, c % 2
        outf[bi, u * NQ:(u + 1) * NQ, :] = res.results[c]["out"]
    if _trace:
        return outf, res
    return outf



# revision 19
# speedup vs baseline: 1.3518x; 1.3518x over previous
"""KeOps-style multi-head attention (unnormalized-exp softmax) on 8 trn2 cores.

Sharding: core c handles batch bi = c//2 and query rows u*1024..(u+1)*1024
(u = c%2), ALL 8 heads. Output is a pure concat over cores (no reduction).

All matmul operands are bf16 (PE runs 1 cycle/row vs 4 for fp32); every
accumulation (PSUM), the softmax denominator path and the final output stay
fp32, keeping the end-to-end rel error ~5e-3 (gate is 2e-2).

Per-core pipeline (one uniform SPMD program):
  A) Batched bf16 DMAs (x via the Activation HWDGE queue, weights/xq via SP);
     transpose x on PE (128x128 identity-matmul transposes, bf16 1 cyc/row),
     8 transposes batched per DVE evacuation.
  B) QKV projections producing q^T/k^T in "stacked head" layout
     [32*h_local + d, n] (heads packed 4-per-tensor so the K=32 scores
     matmuls can be row-tiled on the PE array), and v in normal layout with a
     ones-column appended (the softmax denominator falls out of the same
     matmul that computes the numerator). Only the i=0 half and v chunks
     0..7 are projected in the prologue; the rest interleave into attention
     blocks 1-2 just-in-time on borrowed score-PSUM slots.
  C) Attention over 4 blocks (2 query groups x 2 head quads), software-
     pipelined by one key chunk: scores(j) [4x row-tiled K=32 bf16 matmuls,
     each into its own PSUM bank] -> exp(j) on ACT (psum -> sbuf bf16,
     [128,1024] per instruction) -> numer(j-1) accumulation (e^T moving).
     Numerator accumulators pack two heads per PSUM bank ([97, 512], heads
     at partitions 0/64 - the hardware overlaps those column-group-disjoint
     matmuls) and are double-buffered, so evacuation (DVE + DMA assembly of
     PT) spreads one-copy-per-j through the NEXT block with no boundary
     stall. exp can optionally run partially on DVE via a one-instruction
     Schraudolph bf16 exp (DVE_PICK residues); with the PE clock-limited it
     buys nothing, so it is off for the better accuracy.
  D) Per-query-group epilogue fully overlapped with the next group's
     compute: reciprocal of the denominators in 4 small DVE chunks,
     partition-broadcast via a DRAM bounce, normalize multiplies on the
     otherwise-idle GpSimd (emits bf16 PT), and the output projection on
     borrowed score-PSUM slots; only the second group's short chain remains
     as the tail.

Known limit: the attention loop's matmuls (32-row scores, 33-col numerators)
never look "busy" to the PE_HAM activity monitor, so the PE clock-gate sits
at K=4/8 (1.2 GHz) for most of the loop; full-array dummy work flips it 8/8
only in bursts and costs more than it buys. The loop is therefore PE-bound
at ~537ns per concurrent matmul pair.
"""

import numpy as np
import ml_dtypes
from contextlib import ExitStack

import concourse.bass as bass
import concourse.mybir as mybir
import concourse.tile as tile
from concourse import bacc
from concourse.bass_utils import run_bass_kernel_spmd

DIM = 256
NUM_HEADS = 8
HEAD_DIM = 32
B = 4
N = 2048
NQ = 1024          # query rows per core
NCORES = 8
FP = mybir.dt.float32
BF = mybir.dt.bfloat16
EXP = mybir.ActivationFunctionType.Exp
CPY = mybir.ActivationFunctionType.Copy
LN = mybir.ActivationFunctionType.Ln
I16 = mybir.dt.int16
MULT = mybir.AluOpType.mult
ADD = mybir.AluOpType.add

NT_KV = N // 128   # 16 n-tiles of kv rows
NT_Q = NQ // 128   # 8 n-tiles of q rows
NGQ = NQ // 512    # 2 groups of 512 query cols in q^T
NJ = N // 128      # 16 key chunks of 128
DUMMY_MM_ROWS = 0  # rows of full-array dummy matmul per j (0=off; HAM
DUMMY_EVERY = 2    # experiment knob - net loss, the clock boost never paid)
SCORE_MODE = "full128"  # "pair32": original 32-row row-tiled pairs (HAM
                      #   throttles the PE clock to 4/8 for the whole loop);
                      # "full128": kT whole tile as lhsT vs zero-padded
                      #   per-head q rhs - math identical, HAM sees full-
                      #   array matmuls and holds K=8/8 (-14us) but the 4
                      #   score matmuls serialize;
                      # "pad64": 64-row bands (head + zero-padded sibling),
                      #   two matmuls pair on disjoint row halves -> both
                      #   overlap AND (hopefully) enough activity for K=8/8.
DVE_PICK = set()   # (2j+p)%8 residues whose exp runs on DVE (Schraudolph)
SCHRAU_A = float(2 ** 7 / np.log(2.0))
SCHRAU_B = float(127 * 2 ** 7 - 8.0)


def build_program():
    nc = bacc.Bacc()

    xq = nc.declare_dram_parameter("xq", [NQ, DIM], BF, isOutput=False)
    xkv = nc.declare_dram_parameter("xkv", [N, DIM], BF, isOutput=False)
    wq = nc.declare_dram_parameter("wq", [DIM, DIM], BF, isOutput=False)
    wk = nc.declare_dram_parameter("wk", [DIM, DIM], BF, isOutput=False)
    wv = nc.declare_dram_parameter("wv", [DIM, DIM], BF, isOutput=False)
    wout = nc.declare_dram_parameter("wout", [DIM, DIM], BF, isOutput=False)
    bout = nc.declare_dram_parameter("bout", [DIM], FP, isOutput=False)
    ident_in = nc.declare_dram_parameter("ident", [128, 128], BF, isOutput=False)
    out = nc.declare_dram_parameter("out", [NQ, DIM], FP, isOutput=True)

    with tile.TileContext(nc) as tc, ExitStack() as ctx:
        consts = ctx.enter_context(tc.tile_pool(name="consts", bufs=1))
        persist = ctx.enter_context(tc.tile_pool(name="persist", bufs=1))

        ident = consts.tile([128, 128], BF)
        nc.sync.dma_start(out=ident, in_=ident_in[:, :])
        bias_b = consts.tile([128, DIM], FP)

        # ---- x loads: 8-block batched DMAs (the SP sequencer issues DMA
        # instructions at ~600ns each, so fewer+bigger wins; xkv rides the
        # Activation queue, which is also HWDGE-capable and idle here; xq
        # goes ahead of the weights because the q projection gates the loop
        # start) ----
        xkv_sb = [persist.tile([128, 4, DIM], BF, tag=f"xkvb{b}", name=f"xkvb{b}")
                  for b in range(4)]
        for b in range(4):
            nc.scalar.dma_start(
                out=xkv_sb[b],
                in_=xkv[512 * b:512 * (b + 1), :]
                .rearrange("(t p) d -> p t d", p=128))
        # ---- weights (wq/wk first: the q/k projections gate the loop
        # start; bias is only needed by the output projection) ----
        wq_sb = consts.tile([128, 2, DIM], BF)
        wk_sb = consts.tile([128, 2, DIM], BF)
        wv_sb = consts.tile([128, 2, DIM], BF)
        wout_sb = consts.tile([128, 2, DIM], BF)
        for w_sb, w_d in ((wq_sb, wq), (wk_sb, wk)):
            nc.sync.dma_start(
                out=w_sb, in_=w_d[:, :].rearrange("(ck p) d -> p ck d", p=128))
        xq_sb = [persist.tile([128, 4, DIM], BF, tag=f"xqb{b}", name=f"xqb{b}")
                 for b in range(2)]
        for b in range(2):
            nc.sync.dma_start(
                out=xq_sb[b],
                in_=xq[512 * b:512 * (b + 1), :]
                .rearrange("(t p) d -> p t d", p=128))
        for w_sb, w_d in ((wv_sb, wv), (wout_sb, wout)):
            nc.sync.dma_start(
                out=w_sb, in_=w_d[:, :].rearrange("(ck p) d -> p ck d", p=128))
        nc.sync.dma_start(out=bias_b, in_=bout[:].unsqueeze(0).to_broadcast([128, DIM]))
        # Warm the ACT table RAM with the exp set during the prologue so
        # the first real exp doesn't stall ~2.7us on ACT_TABLE_LOAD.
        tbl_warm = consts.tile([1, DIM], FP)
        nc.scalar.activation(tbl_warm, bias_b[0:1, :], EXP)

        # ---- transposes: xkvT [128(c_local), ck, n], xqT [128, ck, nq] ----
        # 8 transposes share one [128, 1024] bf16 psum tile (sequential
        # writes to one bank), evacuated by a single DVE copy.
        xkvT = persist.tile([128, 2, N], BF)
        xqT = persist.tile([128, 2, NQ], BF)
        with tc.tile_pool(name="tps", bufs=3, space="PSUM") as tps:
            for ck in range(2):
                for b8 in range(NT_KV // 8):
                    ps = tps.tile([128, 1024], BF, tag="tp")
                    for k in range(8):
                        t = 8 * b8 + k
                        nc.tensor.transpose(
                            ps[:, 128 * k:128 * (k + 1)],
                            xkv_sb[t // 4][:, t % 4, 128 * ck:128 * (ck + 1)], ident)
                    nc.vector.tensor_copy(
                        xkvT[:, ck, 1024 * b8:1024 * (b8 + 1)], ps)
                ps = tps.tile([128, 1024], BF, tag="tp")
                for k in range(NT_Q):
                    nc.tensor.transpose(
                        ps[:, 128 * k:128 * (k + 1)],
                        xq_sb[k // 4][:, k % 4, 128 * ck:128 * (ck + 1)], ident)
                nc.vector.tensor_copy(xqT[:, ck, :], ps)

        # ---- QKV projections ----
        # qT/kT stacked-head layout: tensor i in {0,1} holds heads 4i..4i+3:
        # row 32*hloc + d  <->  head 4i+hloc, dim d.
        qT = [persist.tile([128, NQ], BF, tag=f"qT{i}", name=f"qT{i}") for i in range(2)]
        kT = [persist.tile([128, N], BF, tag=f"kT{i}", name=f"kT{i}") for i in range(2)]
        # v normal layout + ones column: [128(n), t, h, 33]
        v_sb = persist.tile([128, NT_KV, NUM_HEADS, HEAD_DIM + 1], BF)
        nc.vector.memset(v_sb[:, :, :, HEAD_DIM:], 1.0)

        with (
            tc.tile_pool(name="qkvp", bufs=3, space="PSUM") as qkvp,
            tc.tile_pool(name="vp", bufs=2, space="PSUM") as vp,
        ):
            for i in range(1):
                for g in range(NGQ):
                    ps = qkvp.tile([128, 512], FP, tag="proj")
                    for ck in range(2):
                        nc.tensor.matmul(
                            ps, lhsT=wq_sb[:, ck, 128 * i:128 * (i + 1)],
                            rhs=xqT[:, ck, 512 * g:512 * (g + 1)],
                            start=(ck == 0), stop=(ck == 1))
                    nc.vector.tensor_copy(qT[i][:, 512 * g:512 * (g + 1)], ps)
                for g in range(N // 512):
                    ps = qkvp.tile([128, 512], FP, tag="proj")
                    for ck in range(2):
                        nc.tensor.matmul(
                            ps, lhsT=wk_sb[:, ck, 128 * i:128 * (i + 1)],
                            rhs=xkvT[:, ck, 512 * g:512 * (g + 1)],
                            start=(ck == 0), stop=(ck == 1))
                    nc.vector.tensor_copy(kT[i][:, 512 * g:512 * (g + 1)], ps)
            for t in range(NT_KV // 2):
                ps = vp.tile([128, DIM], FP, tag="vproj")
                for ck in range(2):
                    nc.tensor.matmul(
                        ps, lhsT=xkvT[:, ck, 128 * t:128 * (t + 1)],
                        rhs=wv_sb[:, ck, :],
                        start=(ck == 0), stop=(ck == 1))
                # strided copy into the 33-wide per-head slots, on the Scalar
                # engine (idle during the prologue; DVE handles q/k evacs)
                nc.scalar.activation(v_sb[:, t, :, 0:HEAD_DIM], ps, CPY)

        # ---- attention ----
        # PT: normalized pre-projection, transposed: tensor i rows = wout rows
        # 128i..128i+128 (head 4i+hloc dim d at partition 32*hloc+d).
        PTf = [persist.tile([128, NQ], FP, tag=f"PTf{i}", name=f"PTf{i}") for i in range(2)]
        PTb = [persist.tile([128, NQ], BF, tag=f"PTb{i}", name=f"PTb{i}") for i in range(2)]
        # per-(g,hh) denominators ([4, 512], row hloc): separate tiles so
        # each hh half can be reciprocal'd/broadcast as soon as its own
        # evac lands (engine reads must start at an aligned partition, so
        # row-slices of one bigger tile won't do).
        denoms = [[persist.tile([4, 512], FP, tag=f"dn{g}{hh}", name=f"dn{g}{hh}")
                   for hh in range(2)] for g in range(NGQ)]
        recips = [[persist.tile([4, 512], FP, tag=f"rc{g}{hh}", name=f"rc{g}{hh}")
                   for hh in range(2)] for g in range(NGQ)]
        rb = [persist.tile([128, NQ], FP, tag=f"rb{i}", name=f"rb{i}") for i in range(2)]
        if SCORE_MODE != "pair32":
            # per-(hh,hloc) zero-padded q: only rows holding head 4hh+hloc
            # are live (refreshed per g); the zero rows make a wider-K
            # matmul against a taller kT slice compute exactly that head.
            # full128: one [128,512] tile per hloc (band at 32*hloc).
            # pad64: one [128,512] tile per uu; rows 0:64 serve hloc=uu
            # (band at 32*uu) and rows 64:128 serve hloc=2+uu (band at
            # 64+32*uu) - rhs slices start at the same partition as the
            # kT lhsT slice (a hard PE requirement).
            nqp = 4 if SCORE_MODE == "full128" else 2
            qpad = [[persist.tile([128, 512], BF, tag=f"qp{hh}{hl}",
                                  name=f"qp{hh}{hl}") for hl in range(nqp)]
                    for hh in range(2)]
            for hh in range(2):
                for hl in range(nqp):
                    nc.vector.memset(qpad[hh][hl], 0.0)

        with (
            tc.tile_pool(name="spsum", bufs=2, space="PSUM") as spsum,
            tc.tile_pool(name="npsum", bufs=2, space="PSUM") as npsum,
            tc.tile_pool(name="esb", bufs=4) as esb,
            tc.tile_pool(name="evac", bufs=4) as evac,
            tc.tile_pool(name="osb", bufs=4) as osb,
            tc.tile_pool(name="dscratch", bufs=2, space="DRAM") as dsc,
        ):
            # Numerator accumulators pack two heads per PSUM bank ([97, 512]:
            # head pair at partition 0 and 64, both 32-aligned), so a double-
            # buffered pair of tiles fits beside the score tiles.  With
            # bufs=2 the evacuation of block b overlaps block b+1 freely.
            def outproj_t(t, pool):
                if pool is spsum:
                    ps = pool.tile([128, 1024], FP, tag="sp", name=f"op{t}")
                else:
                    ps = pool.tile([128, DIM], FP, tag="o", name=f"op{t}")
                pso = ps[:, 0:DIM]
                for i in range(2):
                    nc.tensor.matmul(
                        pso, lhsT=PTb[i][:, 128 * t:128 * (t + 1)],
                        rhs=wout_sb[:, i, :],
                        start=(i == 0), stop=(i == 1))
                ob = osb.tile([128, DIM], FP, tag="ob")
                nc.vector.tensor_add(ob, pso, bias_b)
                nc.sync.dma_start(out=out[128 * t:128 * (t + 1), :], in_=ob)

            # Side work is spread one closure per j so no engine queue
            # ever receives a multi-us blob ahead of exp work it gates.
            def evac_one(g, hh, nptiles, hloc, on_act=False, tail=False):
                def f():
                    npt = nptiles[hloc // 2]
                    r0 = 64 * (hloc % 2)
                    tmp = evac.tile([HEAD_DIM + 1, 512], FP, tag="ev")
                    if on_act:
                        nc.scalar.activation(tmp, npt[r0:r0 + HEAD_DIM + 1, :],
                                             CPY)
                    else:
                        nc.vector.tensor_copy(tmp, npt[r0:r0 + HEAD_DIM + 1, :])
                    nc.sync.dma_start(
                        out=PTf[hh][32 * hloc:32 * hloc + 32,
                                    512 * g:512 * (g + 1)],
                        in_=tmp[0:HEAD_DIM, :])
                    # in the tail the GpSimd queue is idle - issue the denom
                    # row DMAs there so the SP queue only carries the PTf rows
                    eng = nc.gpsimd if tail else nc.sync
                    eng.dma_start(
                        out=denoms[g][hh][hloc:hloc + 1, :],
                        in_=tmp[HEAD_DIM:HEAD_DIM + 1, :])
                return f

            def recip_hh(g, hh):
                # ~18 correct bits, 5x faster than the Newton reciprocal()
                # (denominators are sums of exps, comfortably inside range)
                def f():
                    nc.vector.reciprocal_approx_fast(recips[g][hh],
                                                     denoms[g][hh])
                return f

            def bcast_hh(g, hh, tail=False):
                # In the tail, half the broadcasts issue from the ACT HWDGE
                # queue in parallel with the SP queue (~590ns sequencer issue
                # per dma_start dominates the tail chain). Mid-loop
                # they stay on SP: the ACT queue is busy with exps there.
                def f():
                    rd = dsc.tile([4, 512], FP, tag="rd")
                    nc.sync.dma_start(out=rd[:, :], in_=recips[g][hh])
                    for hloc in range(4):
                        eng = nc.scalar if (tail and hloc % 2 == 0) else nc.sync
                        eng.dma_start(
                            out=rb[hh][32 * hloc:32 * hloc + 32,
                                       512 * g:512 * (g + 1)],
                            in_=rd[hloc:hloc + 1, :]
                            .to_broadcast([32, 512]))
                return f

            def norm_mul(g, hh):
                def f():
                    nc.gpsimd.tensor_mul(
                        PTb[hh][:, 512 * g:512 * (g + 1)],
                        PTf[hh][:, 512 * g:512 * (g + 1)],
                        rb[hh][:, 512 * g:512 * (g + 1)])
                return f

            def oproj(t):
                def f():
                    outproj_t(t, spsum)
                return f

            def proj1(kind, g):
                # i=1 q/k projection group on a borrowed sp PSUM slot:
                # real full-array PE work interleaved into blocks 1-2
                # just-in-time (shorter prologue, and the HAM activity
                # monitor sees a busy full array -> clock stays up longer)
                def f():
                    ps = spsum.tile([128, 1024], FP, tag="sp", name=f"pj{kind}{g}")
                    pso = ps[:, 0:512]
                    w_sb, dstT, srcT = ((wq_sb, qT, xqT) if kind == "q"
                                        else (wk_sb, kT, xkvT))
                    for ck in range(2):
                        nc.tensor.matmul(
                            pso, lhsT=w_sb[:, ck, 128:256],
                            rhs=srcT[:, ck, 512 * g:512 * (g + 1)],
                            start=(ck == 0), stop=(ck == 1))
                    nc.vector.tensor_copy(dstT[1][:, 512 * g:512 * (g + 1)], pso)
                return f

            def vpair(t):
                # v projection for chunks t, t+1 on a borrowed sp PSUM slot
                def f():
                    ps = spsum.tile([128, 1024], FP, tag="sp", name=f"vp{t}")
                    for w in range(2):
                        pso = ps[:, 256 * w:256 * (w + 1)]
                        for ck in range(2):
                            nc.tensor.matmul(
                                pso,
                                lhsT=xkvT[:, ck, 128 * (t + w):128 * (t + w + 1)],
                                rhs=wv_sb[:, ck, :],
                                start=(ck == 0), stop=(ck == 1))
                    nc.scalar.activation(v_sb[:, t, :, 0:HEAD_DIM],
                                         ps[:, 0:256], CPY)
                    nc.vector.tensor_copy(v_sb[:, t + 1, :, 0:HEAD_DIM],
                                          ps[:, 256:512])
                return f

            sidework = {
                (0, 0): [None] * 16,
                (0, 1): [None] * 16,
                # during (g1, hh0): evac (g0,hh1); normalize g0; outproj g0
                (1, 0): [None] * 16,
                (1, 1): [None] * 16,
            }
            w00 = sidework[(0, 0)]
            w00[1] = proj1("q", 0)
            w00[3] = proj1("q", 1)
            w00[5] = proj1("k", 0)
            for c in range(4):
                w00[2 * c + 2] = vpair(8 + 2 * c)
            w01 = sidework[(0, 1)]
            w01[1] = proj1("k", 1)
            w01[4] = proj1("k", 2)
            w01[8] = proj1("k", 3)
            for g in range(NGQ):
                for hh in range(2):
                    if SCORE_MODE == "full128":
                        for hl in range(4):
                            nc.vector.tensor_copy(
                                qpad[hh][hl][32 * hl:32 * hl + 32, :],
                                qT[hh][32 * hl:32 * hl + 32,
                                       512 * g:512 * (g + 1)])
                    elif SCORE_MODE == "pad64":
                        for hl in range(4):
                            uu = hl % 2
                            r0 = 64 * (hl // 2) + 32 * uu
                            nc.vector.tensor_copy(
                                qpad[hh][uu][r0:r0 + 32, :],
                                qT[hh][32 * hl:32 * hl + 32,
                                       512 * g:512 * (g + 1)])
                    nptiles = [npsum.tile([97, 512], FP, tag=f"np{x}",
                                          name=f"np{x}") for x in range(2)]

                    def numers(j, es):
                        for p in range(2):
                            for uu in range(2):
                                hloc = 2 * p + uu
                                h = 4 * hh + hloc
                                npt = nptiles[hloc // 2]
                                r0 = 64 * (hloc % 2)
                                nc.tensor.matmul(
                                    npt[r0:r0 + HEAD_DIM + 1, :],
                                    lhsT=v_sb[:, j, h, :],
                                    rhs=es[p][:, 512 * uu:512 * (uu + 1)],
                                    start=(j == 0), stop=(j == NJ - 1))

                    # Software-pipelined by one j: scores(j) [4x row-tiled,
                    # all concurrent] -> exp(j) -> numer(j-1), so the PE
                    # fills the exp latency with independent score work.
                    # exp runs on ACT except for DVE_PICK residues, which use
                    # a one-instruction Schraudolph exp on DVE:
                    #   bf16(exp(x)) ~= bitcast_bf16(int16(x*128/ln2 + Bc))
                    prev = None
                    work = sidework.get((g, hh))
                    for j in range(NJ):
                        sps = []
                        for p in range(2):
                            sp = spsum.tile([128, 1024], FP, tag="sp")
                            if p == 0 and DUMMY_MM_ROWS and j % DUMMY_EVERY == 0:
                                nc.tensor.matmul(
                                    sp[:, 0:DUMMY_MM_ROWS],
                                    lhsT=wout_sb[:, 0, 0:128],
                                    rhs=xkvT[:, 0, 0:DUMMY_MM_ROWS],
                                    start=True, stop=True)
                            sps.append(sp)
                        # pad64: emit in hloc order 0,2,1,3 so adjacent
                        # matmuls sit on disjoint PE row halves and pair.
                        hlocs = (0, 2, 1, 3) if SCORE_MODE == "pad64" else (0, 1, 2, 3)
                        for hloc in hlocs:
                            p, uu = hloc // 2, hloc % 2
                            sp = sps[p]
                            r = 32 * hloc
                            if SCORE_MODE == "full128":
                                nc.tensor.matmul(
                                    sp[:, 512 * uu:512 * (uu + 1)],
                                    lhsT=kT[hh][:, 128 * j:128 * (j + 1)],
                                    rhs=qpad[hh][hloc],
                                    start=True, stop=True)
                            elif SCORE_MODE == "pad64":
                                r64 = 64 * (hloc // 2)
                                nc.tensor.matmul(
                                    sp[:, 512 * uu:512 * (uu + 1)],
                                    lhsT=kT[hh][r64:r64 + 64,
                                                128 * j:128 * (j + 1)],
                                    rhs=qpad[hh][uu][r64:r64 + 64, :],
                                    start=True, stop=True,
                                    tile_position=(r64, 0))
                            else:
                                nc.tensor.matmul(
                                    sp[:, 512 * uu:512 * (uu + 1)],
                                    lhsT=kT[hh][r:r + 32, 128 * j:128 * (j + 1)],
                                    rhs=qT[hh][r:r + 32, 512 * g:512 * (g + 1)],
                                    start=True, stop=True,
                                    tile_position=(r, 0))
                        es = []
                        for p in range(2):
                            e = esb.tile([128, 1024], BF, tag="e")
                            if (2 * j + p) % 8 in DVE_PICK:
                                nc.vector.tensor_scalar(
                                    out=e.bitcast(I16), in0=sps[p],
                                    scalar1=SCHRAU_A, scalar2=SCHRAU_B,
                                    op0=MULT, op1=ADD)
                            else:
                                nc.scalar.activation(e, sps[p], EXP)
                            es.append(e)
                        if prev is not None:
                            numers(*prev)
                        if work is not None and work[j] is not None:
                            work[j]()
                        prev = (j, es)
                    numers(*prev)
                    if (g, hh) == (0, 0):
                        # block (0,1) hosts evac(0,0) AND the whole hh=0
                        # normalize of g=0 (its denoms are complete here)
                        w = sidework[(0, 1)]
                        for slot, k in zip((0, 2, 3, 5), range(4)):
                            w[slot] = evac_one(0, 0, nptiles, k)
                        w[7] = recip_hh(0, 0)
                        w[9] = bcast_hh(0, 0)
                        w[11] = norm_mul(0, 0)
                    elif (g, hh) == (0, 1):
                        # block (1,0): finish g=0 (hh=1 normalize) and run
                        # ALL four g=0 output projections in-loop
                        w = sidework[(1, 0)]
                        for k in range(4):
                            w[k] = evac_one(0, 1, nptiles, k)
                        w[5] = recip_hh(0, 1)
                        w[6] = bcast_hh(0, 1)
                        w[8] = norm_mul(0, 1)
                        w[10] = oproj(0)
                        w[12] = oproj(1)
                        w[13] = oproj(2)
                        w[15] = oproj(3)
                    elif (g, hh) == (1, 0):
                        # block (1,1): evac(1,0) plus the hh=0 half of the
                        # g=1 normalize, so the tail only owns hh=1
                        w = sidework[(1, 1)]
                        for k in range(4):
                            w[k] = evac_one(1, 0, nptiles, k)
                        w[5] = recip_hh(1, 0)
                        w[7] = bcast_hh(1, 0)
                        w[9] = norm_mul(1, 0)
                    else:
                        last_np = nptiles
            # tail: last quad's evac + hh=1 normalize + remaining outproj.
            # (An ACT ln->exp reciprocal was tried here and reverted:
            # having Ln in the program forces extra ACT table loads that
            # eat the gain.)
            for k in range(4):
                evac_one(1, 1, last_np, k, on_act=(k % 2 == 0), tail=True)()
            recip_hh(1, 1)()
            bcast_hh(1, 1, tail=True)()
            nc.vector.tensor_mul(PTb[1][:, 512:1024],
                                 PTf[1][:, 512:1024], rb[1][:, 512:1024])
            for t in range(4, NT_Q):
                outproj_t(t, spsum)
    if not nc.is_finalized():
        nc.finalize()
    return nc


_NC_CACHE = None


def _get_program():
    global _NC_CACHE
    if _NC_CACHE is None:
        _NC_CACHE = build_program()
    return _NC_CACHE


def kernel(x, Wqkv, Wout, bout, _trace=False, _trace_kwargs=None):
    bf = ml_dtypes.bfloat16
    x = np.asarray(x, dtype=np.float32)
    Wqkv = np.asarray(Wqkv, dtype=np.float32)
    Wout = np.asarray(Wout, dtype=np.float32)
    bout = np.asarray(bout, dtype=np.float32)

    scale = HEAD_DIM ** -0.5
    wq = np.ascontiguousarray(Wqkv[:, 0:DIM] * scale).astype(bf)
    wk = np.ascontiguousarray(Wqkv[:, DIM:2 * DIM]).astype(bf)
    wv = np.ascontiguousarray(Wqkv[:, 2 * DIM:3 * DIM]).astype(bf)
    wo = np.ascontiguousarray(Wout).astype(bf)
    xb = x.astype(bf)

    in_maps = []
    for c in range(NCORES):
        bi, u = c // 2, c % 2
        in_maps.append({
            "xq": np.ascontiguousarray(xb[bi, u * NQ:(u + 1) * NQ, :]),
            "xkv": np.ascontiguousarray(xb[bi]),
            "wq": wq, "wk": wk, "wv": wv,
            "wout": wo,
            "bout": bout,
            "ident": np.eye(128, dtype=np.float32).astype(bf),
        })

    nc = _get_program()
    kwargs = {}
    if _trace:
        kwargs["trace"] = True
        if _trace_kwargs:
            kwargs.update(_trace_kwargs)
    res = run_bass_kernel_spmd(nc, in_maps, core_ids=list(range(NCORES)), **kwargs)

    outf = np.empty((B, N, DIM), dtype=np.float32)
    for c in range(NCORES):
        bi, u = c // 2, c % 2
        outf[bi, u * NQ:(u + 1) * NQ, :] = res.results[c]["out"]
    if _trace:
        return outf, res
    return outf

